# revision 23
# baseline (speedup 1.0000x reference)
"""Bass/Trainium2 SPMD kernel for DemopackDecoder (vq_codebook).

Math: decoded[t] = mean_k codewords[indices[t,:,k]]        [512, 4096]
      W[t]       = (decoded[t] @ rotations[t]) * scales[t] [512, 4096]
      out        = x @ concat_t(W[t]).T + bias             [512, 4096]

Sharding (8 cores, expert-parallel over tiles t): core t computes the
output column block [512 tok, 512 feat] for its tile; host concatenates.

Device dataflow (v4 — PE runs only the two GEMMs; decode rides DMA):
  out_t = x @ rot_t^T @ dec_t^T          (rot pre-scaled by s_t/4 on host)
  G1: zT[d,n] = rot_t^T-blocks @ xT[e',n]  -- rot^T blocks (1MB fp16)
      stream from HBM on the sync queue as the stationary operand; x^T is
      SBUF-resident fp16.  rm[0] leads the sync queue (it gates the first
      LDWEIGHTS, half on each hwdge queue) and x chunks stream on the
      scalar queue in exactly m=0's consumption order, so the PE starts
      ~10us in and never waits for x afterwards.
  A : decode rides DMA + DVE, time-gated behind the rm stream: each
      half-row-block indirect gather (deduped codebook, fp16) reads its
      index column through a DVE chain that data-depends on zT[m_g], so
      gathers release ~1 per 1.7 m-blocks; DVE adds form the mean-of-4;
      DMA xbar transposes (16x128 crossbar, SBUF->SBUF) emit decT
      [e'-part, r] with zero PE involvement. All hidden under G1.
  G2: out[r,n] += decT[e,r-block]^T @ zT[e,n]  (128 MMs, j-outer so each
      output tile's bias+store overlaps the next chain)
  C : + bias (DVE, PSUM-read), stores split across both hwdge queues.
All matmul operands fp16 (1 PE cycle/row, ~215ns/MM measured = stream
roofline); PSUM fp32; psZ 6 banks absorb zT-copy jitter, psO 2.
Measured ~290-310us on HW (NTFF) vs 444775ns baseline.
"""

import contextlib

import numpy as np

import concourse.bass as bass
import concourse.mybir as mybir
import concourse.tile as tile
from concourse import bacc, bass_utils

# Problem shapes (hardcoded per contract)
T, R, K, D = 8, 512, 4, 4096
N_CW, N_TOK, N_CORES = 16384, 512, 8
P = 128
LCW_ROWS = 2048          # padded dedup codebook rows per core
KT = D // P              # 32 contraction (e') tiles
MT = D // P              # 32 rotated-feature (d) tiles
RT = R // P              # 4 decoded row tiles
JT = R // P              # 4 local out-feature tiles
XQ = 8                   # x loaded in 8 chunks of 4 k-blocks

_PROGRAM_CACHE = {}


def _build_program(loop_n=1):
    f32 = mybir.dt.float32
    f16 = mybir.dt.float16
    i32 = mybir.dt.int32

    nc = bacc.Bacc("TRN2", target_bir_lowering=False, debug=False)
    lcw = nc.dram_tensor("lcw", [LCW_ROWS, D], f16, kind="ExternalInput").ap()
    ridx = nc.dram_tensor("ridx", [P, RT * K], i32, kind="ExternalInput").ap()
    rtb = nc.dram_tensor("rtb", [MT * P, D], f16, kind="ExternalInput").ap()
    xT = nc.dram_tensor("xT", [D, N_TOK], f16, kind="ExternalInput").ap()
    biasd = nc.dram_tensor("biasd", [P, JT], f32, kind="ExternalInput").ap()
    outT = nc.dram_tensor("outT", [R, N_TOK], f32, kind="ExternalOutput").ap()

    # DRAM views
    rtb_v = rtb.rearrange("(m p) d -> m p d", p=P)         # [32, 128, 4096]
    xT_v = xT.rearrange("(q j p) n -> q p j n", j=4, p=P)  # [8, 128, 4, 512]
    outT_v = outT.rearrange("(j p) n -> p j n", p=P)       # [128, 4, 512]

    with tile.TileContext(nc) as tc:
        with (
            tc.tile_pool(name="const", bufs=1) as cpool,
            tc.tile_pool(name="xbuf", bufs=XQ) as xpool,
            tc.tile_pool(name="zbuf", bufs=MT) as zpool,
            tc.tile_pool(name="decT", bufs=1) as dpool,
            tc.tile_pool(name="rbuf", bufs=5) as rpool,
            tc.tile_pool(name="accp", bufs=RT) as apool,
            tc.tile_pool(name="outp", bufs=1) as opool,
            tc.tile_pool(name="gate", bufs=4 * RT * K) as gpool,
            tc.tile_pool(name="gsc", bufs=RT) as spool,
            tc.tile_pool(name="psZ", bufs=6, space="PSUM") as psZ,
            tc.tile_pool(name="psO", bufs=2, space="PSUM") as psO,
        ):
            ridx_sb = cpool.tile([P, RT * K], i32, tag="ridx_sb")
            nc.gpsimd.dma_start(ridx_sb[:], ridx)
            bias_sb = cpool.tile([P, JT], f32, tag="bias_sb")
            nc.gpsimd.dma_start(bias_sb[:], biasd)

            loop_cm = tc.For_i(0, loop_n, 1) if loop_n > 1 else contextlib.nullcontext()
            with loop_cm:
                _emit_body(nc, tc, locals())

    nc.compile()
    return nc


def _emit_body(nc, tc, env, phases="g1 dec g2"):
    f32 = mybir.dt.float32
    f16 = mybir.dt.float16
    lcw, ridx_sb, bias_sb = env["lcw"], env["ridx_sb"], env["bias_sb"]
    rtb_v, xT_v, outT_v = env["rtb_v"], env["xT_v"], env["outT_v"]
    xpool, zpool, dpool, rpool = env["xpool"], env["zpool"], env["dpool"], env["rpool"]
    apool, opool, gpool = env["apool"], env["opool"], env["gpool"]
    spool = env["spool"]
    psZ, psO = env["psZ"], env["psO"]

    # resident SBUF tiles
    xsb = [xpool.tile([P, 4 * N_TOK], f16, tag="xsb", name=f"xsb{q}") for q in range(XQ)]
    zT = [zpool.tile([P, N_TOK], f16, tag="zT", name=f"zT{m}") for m in range(MT)]
    # decT packed as one tile: decT[kk] = cols [kk*512, kk*512+512)
    decT = dpool.tile([P, KT * R], f16, tag="decT")
    decT_v = decT[:].rearrange("p (s x) -> p s x", x=R)    # [128, 32, 512]

    dec_on = "dec" in phases

    # The rm stream needs ~154 GB/s sustained; one hwdge queue tops out
    # near that, so rm tiles ALTERNATE between the two queues (77 GB/s
    # each). rm[0] is split across both (it gates the first LDWEIGHTS);
    # rm[1] is dispatched ahead of the x chunks; x chunks alternate too
    # so m=0 never waits on a single-queue x backlog.
    rm0 = rpool.tile([P, D], f16, tag="rm")
    rm1 = rpool.tile([P, D], f16, tag="rm")
    nc.sync.dma_start(rm0[:, :D // 2], rtb_v[0][:, :D // 2])
    nc.scalar.dma_start(
        xsb[0][:].rearrange("p (j n) -> p j n", n=N_TOK), xT_v[0])
    nc.scalar.dma_start(rm0[:, D // 2:], rtb_v[0][:, D // 2:])
    nc.sync.dma_start(rm1[:], rtb_v[1])
    for q in range(1, XQ):
        eng = nc.sync if q % 2 == 1 else nc.scalar
        eng.dma_start(
            xsb[q][:].rearrange("p (j n) -> p j n", n=N_TOK), xT_v[q]
        )

    # decode: per row-block i, 4 gathers accumulate into acc via the DMA
    # compute engine; then 4 xbar transposes (e'-quarters) build decT.
    # Each gather's index column is routed through a DVE chain that
    # data-depends on zT[m_g], time-gating gathers to ~1 per 1.7 m-blocks
    # so the rm stream never starves on the DMA engines.
    accs, gscs, gates = [], [], {}
    if dec_on:
        for i in range(RT):
            accs.append(apool.tile([P, D], f16, tag="acc", name=f"acc{i}"))
            gscs.append(spool.tile([P, D // 2], f16, tag="gsc", name=f"gsc{i}"))
        for i in range(RT):
            for k in range(K):
                for h in range(2):
                    gates[(i, k, h)] = (
                        gpool.tile([P, 1], mybir.dt.int32, tag="gate",
                                   name=f"gate{i}_{k}_{h}"),
                        gpool.tile([P, 1], mybir.dt.float32, tag="gatez",
                                   name=f"gatez{i}_{k}_{h}"),
                    )
    # decode work queue: per group i, 8 half-gathers (2.9us -> 1.45us DMA
    # bursts) + 4 xbar transposes, drained ~1.8 ops per m-block via the
    # zT gate chain so DMA demand stays smooth next to the rm stream.
    H = D // 2
    decode_ops = []
    for i in range(RT):
        for k in range(K):
            for h in range(2):
                decode_ops.append(("g", i, k, h))
        for eq in range(4):
            decode_ops.append(("t", i, eq))
    gate_at = {}
    for jop, op in enumerate(decode_ops):
        gate_at.setdefault(2 + (27 * jop) // len(decode_ops), []).append(op)

    def emit_decode(op, m):
        if op[0] == "g":
            _, i, k, h = op
            # gated index column: gz = 0*zT[m] (data dep on block m),
            # gt = ridx + gz — the gather can't start before m-block m.
            gt, gz = gates[(i, k, h)]
            nc.vector.tensor_scalar(
                out=gz[:], in0=zT[m][:, 0:1], scalar1=0.0, scalar2=None,
                op0=mybir.AluOpType.mult)
            nc.vector.tensor_scalar(
                out=gt[:], in0=ridx_sb[:, i * K + k: i * K + k + 1],
                scalar1=gz[:], scalar2=None, op0=mybir.AluOpType.add)
            if k == 0:
                nc.gpsimd.indirect_dma_start(
                    out=accs[i][:, h * H:(h + 1) * H], out_offset=None,
                    in_=lcw, element_offset=h * H,
                    in_offset=bass.IndirectOffsetOnAxis(ap=gt[:], axis=0),
                )
            else:
                nc.gpsimd.indirect_dma_start(
                    out=gscs[i][:], out_offset=None,
                    in_=lcw, element_offset=h * H,
                    in_offset=bass.IndirectOffsetOnAxis(ap=gt[:], axis=0),
                )
                nc.vector.tensor_add(
                    accs[i][:, h * H:(h + 1) * H],
                    accs[i][:, h * H:(h + 1) * H],
                    gscs[i][:])
        else:
            _, i, eq = op
            nc.scalar.dma_start_transpose(
                decT_v[:, eq * 8:(eq + 1) * 8, i * P:(i + 1) * P],
                accs[i][:, eq * (D // 4):(eq + 1) * (D // 4)],
            )

    # ---- G1: zT[m] = sum_k rtb[m,:,k-block]^T @ xT[k-block,:] ----
    for m in range(MT):
        if m == 0:
            rm = rm0
        elif m == 1:
            rm = rm1
        else:
            rm = rpool.tile([P, D], f16, tag="rm")
            eng = nc.sync if m % 2 == 0 else nc.scalar
            eng.dma_start(rm[:], rtb_v[m])
        ps = psZ.tile([P, N_TOK], f32, tag="psZ")
        for k in range(KT):
            nc.tensor.matmul(
                ps[:],
                lhsT=rm[:, k * P:(k + 1) * P],
                rhs=xsb[k // 4][:, (k % 4) * N_TOK:(k % 4 + 1) * N_TOK],
                start=(k == 0),
                stop=(k == KT - 1),
            )
        nc.vector.tensor_copy(zT[m][:], ps[:])

        if dec_on:
            for op in gate_at.get(m, []):
                emit_decode(op, m)

    if "g2" not in phases:
        # timing probe (g1 / nog2): consume zT (+ decT if present) so
        # nothing can be DCE'd, skip the G2 GEMM
        out_sb = opool.tile([P, JT * N_TOK], f32, tag="osb")
        for m in range(MT):
            nc.vector.tensor_copy(
                out_sb[:, (m % JT) * N_TOK:(m % JT + 1) * N_TOK], zT[m][:])
        if dec_on:
            for j in range(JT):
                nc.vector.tensor_copy(
                    out_sb[:, j * N_TOK:(j + 1) * N_TOK],
                    decT_v[:, j, :])
        nc.sync.dma_start(
            outT_v, out_sb[:].rearrange("p (j n) -> p j n", n=N_TOK))
        return

    # ---- G2: out[r,n] = sum_e decT[e,r]^T @ zT[e,n] ----
    # j-outer so each output tile's bias+store overlaps the next j's MMs
    out_sb = opool.tile([P, JT * N_TOK], f32, tag="osb")
    for j in range(JT):
        out_ps = psO.tile([P, N_TOK], f32, tag="psO", name=f"outps{j}")
        for m in range(MT):
            nc.tensor.matmul(
                out_ps[:],
                lhsT=decT_v[:, m, j * P:(j + 1) * P],
                rhs=zT[m][:],
                start=(m == 0),
                stop=(m == MT - 1),
            )
        nc.vector.tensor_scalar(
            out=out_sb[:, j * N_TOK:(j + 1) * N_TOK],
            in0=out_ps[:],
            scalar1=bias_sb[:, j:j + 1],
            scalar2=None,
            op0=mybir.AluOpType.add,
        )
        half = N_TOK // 2
        nc.sync.dma_start(
            outT_v[:, j, :half], out_sb[:, j * N_TOK:j * N_TOK + half])
        nc.scalar.dma_start(
            outT_v[:, j, half:], out_sb[:, j * N_TOK + half:(j + 1) * N_TOK])


def _get_program(loop_n=1):
    if loop_n not in _PROGRAM_CACHE:
        _PROGRAM_CACHE[loop_n] = _build_program(loop_n)
    return _PROGRAM_CACHE[loop_n]


def _build_bench_program(loop_n, phases="g1 dec g2"):
    """Timing-only variant: big tensors are Internal (device-resident, no
    per-call upload over axon — kills the transfer noise that swamps the
    loop delta) and the body repeats a static loop_n times. Values in
    lcw/rtb/xT are garbage; engine timing is data-independent. ridx stays
    a real external input (it feeds DMA offsets, which must stay
    in-bounds)."""
    f32 = mybir.dt.float32
    f16 = mybir.dt.float16
    i32 = mybir.dt.int32

    nc = bacc.Bacc("TRN2", target_bir_lowering=False, debug=False)
    lcw = nc.dram_tensor("lcw", [LCW_ROWS, D], f16, kind="Internal").ap()
    ridx = nc.dram_tensor("ridx", [P, RT * K], i32, kind="ExternalInput").ap()
    rtb = nc.dram_tensor("rtb", [MT * P, D], f16, kind="Internal").ap()
    xT = nc.dram_tensor("xT", [D, N_TOK], f16, kind="Internal").ap()
    biasd = nc.dram_tensor("biasd", [P, JT], f32, kind="ExternalInput").ap()
    outT = nc.dram_tensor("outT", [R, N_TOK], f32, kind="ExternalOutput").ap()

    rtb_v = rtb.rearrange("(m p) d -> m p d", p=P)
    xT_v = xT.rearrange("(q j p) n -> q p j n", j=4, p=P)
    outT_v = outT.rearrange("(j p) n -> p j n", p=P)

    with tile.TileContext(nc) as tc:
        with (
            tc.tile_pool(name="const", bufs=1) as cpool,
            tc.tile_pool(name="xbuf", bufs=XQ) as xpool,
            tc.tile_pool(name="zbuf", bufs=MT) as zpool,
            tc.tile_pool(name="decT", bufs=1) as dpool,
            tc.tile_pool(name="rbuf", bufs=5) as rpool,
            tc.tile_pool(name="accp", bufs=RT) as apool,
            tc.tile_pool(name="outp", bufs=1) as opool,
            tc.tile_pool(name="gate", bufs=4 * RT * K) as gpool,
            tc.tile_pool(name="gsc", bufs=RT) as spool,
            tc.tile_pool(name="psZ", bufs=6, space="PSUM") as psZ,
            tc.tile_pool(name="psO", bufs=2, space="PSUM") as psO,
        ):
            ridx_sb = cpool.tile([P, RT * K], i32, tag="ridx_sb")
            nc.gpsimd.dma_start(ridx_sb[:], ridx)
            bias_sb = cpool.tile([P, JT], f32, tag="bias_sb")
            nc.gpsimd.dma_start(bias_sb[:], biasd)

            loop_cm = tc.For_i(0, loop_n, 1) if loop_n > 1 else contextlib.nullcontext()
            with loop_cm:
                _emit_body(nc, tc, locals(), phases=phases)

    nc.compile()
    return nc


def _get_bench_program(loop_n, phases="g1 dec g2"):
    key = ("bench", loop_n, phases)
    if key not in _PROGRAM_CACHE:
        _PROGRAM_CACHE[key] = _build_bench_program(loop_n, phases)
    return _PROGRAM_CACHE[key]


def _make_in_maps(x, codewords, indices, rotations, scales, bias):
    x = np.asarray(x, dtype=np.float32)
    codewords = np.asarray(codewords, dtype=np.float32)
    indices = np.asarray(indices)
    rotations = np.asarray(rotations, dtype=np.float32)
    scales = np.asarray(scales, dtype=np.float32)
    bias = np.asarray(bias, dtype=np.float32)

    xTh = np.ascontiguousarray(x.T.astype(np.float16))  # [4096, 512]
    in_maps = []
    for t in range(T):
        idx_t = indices[t].reshape(-1).astype(np.int64)
        uniq, inv = np.unique(idx_t, return_inverse=True)
        assert len(uniq) <= LCW_ROWS
        lcw = np.zeros((LCW_ROWS, D), np.float16)
        lcw[: len(uniq)] = codewords[uniq].astype(np.float16)
        inv = inv.reshape(R, K).astype(np.int32)
        ridx = np.zeros((P, RT * K), np.int32)
        for i in range(RT):
            for k in range(K):
                ridx[:, i * K + k] = inv[i * P:(i + 1) * P, k]
        # G1 contracts over rot's SECOND axis (e'): z[n,d] = sum_e x[n,e]rot[d,e]
        # lhsT must be rot^T: rtb[m, p, k*128+j] = (rot*s/4)^T[k*128+p, m*128+j]
        rt = (rotations[t] * (scales[t] / K)).T
        rtb = np.ascontiguousarray(
            rt.reshape(KT, P, MT, P).transpose(2, 1, 0, 3)
            .reshape(MT * P, D).astype(np.float16)
        )
        bias_t = np.ascontiguousarray(
            bias[R * t: R * (t + 1)].reshape(JT, P).T
        ).astype(np.float32)
        in_maps.append(
            {"lcw": lcw, "ridx": ridx, "rtb": rtb, "xT": xTh, "biasd": bias_t}
        )
    return in_maps


def kernel(x, codewords, indices, rotations, scales, bias):
    in_maps = _make_in_maps(x, codewords, indices, rotations, scales, bias)
    nc = _get_program()
    res = bass_utils.run_bass_kernel_spmd(nc, in_maps, core_ids=list(range(N_CORES)))
    out = np.empty((N_TOK, T * R), np.float32)
    for t in range(T):
        out[:, R * t: R * (t + 1)] = res.results[t]["outT"].T
    return out


if __name__ == "__main__":
    rng = np.random.default_rng(0)
    ins = {
        "x": rng.standard_normal((N_TOK, D), dtype=np.float32),
        "codewords": rng.standard_normal((N_CW, D), dtype=np.float32) * 0.02,
        "indices": rng.integers(0, N_CW, size=(T, R, K)),
        "rotations": rng.standard_normal((T, D, D), dtype=np.float32) / np.sqrt(D),
        "scales": (rng.random(T, dtype=np.float32) + 0.5),
        "bias": np.zeros(D, np.float32),
    }
    out = kernel(**ins)
    print("out", out.shape, out.dtype, np.abs(out).mean())


# revision 24
# speedup vs baseline: 1.0107x; 1.0107x over previous
"""Bass/Trainium2 SPMD kernel for DemopackDecoder (vq_codebook).

Math: decoded[t] = mean_k codewords[indices[t,:,k]]        [512, 4096]
      W[t]       = (decoded[t] @ rotations[t]) * scales[t] [512, 4096]
      out        = x @ concat_t(W[t]).T + bias             [512, 4096]

Sharding (8 cores, expert-parallel over tiles t): core t computes the
output column block [512 tok, 512 feat] for its tile; host concatenates.

Device dataflow (v4 — PE runs only the two GEMMs; decode rides DMA):
  out_t = x @ rot_t^T @ dec_t^T          (rot pre-scaled by s_t/4 on host)
  G1: zT[d,n] = rot_t^T-blocks @ xT[e',n]  -- rot^T blocks (1MB fp16)
      stream from HBM on the sync queue as the stationary operand; x^T is
      SBUF-resident fp16.  rm[0] leads the sync queue (it gates the first
      LDWEIGHTS, half on each hwdge queue) and x chunks stream on the
      scalar queue in exactly m=0's consumption order, so the PE starts
      ~10us in and never waits for x afterwards.
  A : decode rides DMA + DVE, time-gated behind the rm stream: each
      half-row-block indirect gather (deduped codebook, fp16) reads its
      index column through a DVE chain that data-depends on zT[m_g], so
      gathers release ~1 per 1.7 m-blocks; DVE adds form the mean-of-4;
      DMA xbar transposes (16x128 crossbar, SBUF->SBUF) emit decT
      [e'-part, r] with zero PE involvement. All hidden under G1.
  G2: out[r,n] += decT[e,r-block]^T @ zT[e,n]  (128 MMs, j-outer so each
      output tile's bias+store overlaps the next chain)
  C : + bias (DVE, PSUM-read), stores split across both hwdge queues.
All matmul operands fp16 (1 PE cycle/row, ~215ns/MM measured = stream
roofline); PSUM fp32; psZ 6 banks absorb zT-copy jitter, psO 2.
Measured ~290-310us on HW (NTFF) vs 444775ns baseline.
"""

import contextlib

import numpy as np

import concourse.bass as bass
import concourse.mybir as mybir
import concourse.tile as tile
from concourse import bacc, bass_utils

# Problem shapes (hardcoded per contract)
T, R, K, D = 8, 512, 4, 4096
N_CW, N_TOK, N_CORES = 16384, 512, 8
P = 128
LCW_ROWS = 2048          # padded dedup codebook rows per core
KT = D // P              # 32 contraction (e') tiles
MT = D // P              # 32 rotated-feature (d) tiles
RT = R // P              # 4 decoded row tiles
JT = R // P              # 4 local out-feature tiles
XQ = 8                   # x loaded in 8 chunks of 4 k-blocks

_PROGRAM_CACHE = {}


def _build_program(loop_n=1):
    f32 = mybir.dt.float32
    f16 = mybir.dt.float16
    i32 = mybir.dt.int32

    nc = bacc.Bacc("TRN2", target_bir_lowering=False, debug=False)
    lcw = nc.dram_tensor("lcw", [LCW_ROWS, D], f16, kind="ExternalInput").ap()
    ridx = nc.dram_tensor("ridx", [P, RT * K], i32, kind="ExternalInput").ap()
    rtb = nc.dram_tensor("rtb", [MT * P, D], f16, kind="ExternalInput").ap()
    xT = nc.dram_tensor("xT", [D, N_TOK], f16, kind="ExternalInput").ap()
    biasd = nc.dram_tensor("biasd", [P, JT], f32, kind="ExternalInput").ap()
    outT = nc.dram_tensor("outT", [R, N_TOK], f32, kind="ExternalOutput").ap()

    # DRAM views
    rtb_v = rtb.rearrange("(m p) d -> m p d", p=P)         # [32, 128, 4096]
    xT_v = xT.rearrange("(q j p) n -> q p j n", j=4, p=P)  # [8, 128, 4, 512]
    outT_v = outT.rearrange("(j p) n -> p j n", p=P)       # [128, 4, 512]

    with tile.TileContext(nc) as tc:
        with (
            tc.tile_pool(name="const", bufs=1) as cpool,
            tc.tile_pool(name="xbuf", bufs=XQ) as xpool,
            tc.tile_pool(name="zbuf", bufs=MT) as zpool,
            tc.tile_pool(name="decT", bufs=1) as dpool,
            tc.tile_pool(name="rbuf", bufs=5) as rpool,
            tc.tile_pool(name="accp", bufs=RT) as apool,
            tc.tile_pool(name="outp", bufs=1) as opool,
            tc.tile_pool(name="gate", bufs=5 * RT * K) as gpool,
            tc.tile_pool(name="gsc", bufs=RT) as spool,
            tc.tile_pool(name="psZ", bufs=6, space="PSUM") as psZ,
            tc.tile_pool(name="psO", bufs=2, space="PSUM") as psO,
        ):
            ridx_sb = cpool.tile([P, RT * K], i32, tag="ridx_sb")
            nc.gpsimd.dma_start(ridx_sb[:], ridx)
            bias_sb = cpool.tile([P, JT], f32, tag="bias_sb")
            nc.gpsimd.dma_start(bias_sb[:], biasd)

            loop_cm = tc.For_i(0, loop_n, 1) if loop_n > 1 else contextlib.nullcontext()
            with loop_cm:
                _emit_body(nc, tc, locals())

    nc.compile()
    return nc


def _emit_body(nc, tc, env, phases="g1 dec g2"):
    f32 = mybir.dt.float32
    f16 = mybir.dt.float16
    lcw, ridx_sb, bias_sb = env["lcw"], env["ridx_sb"], env["bias_sb"]
    rtb_v, xT_v, outT_v = env["rtb_v"], env["xT_v"], env["outT_v"]
    xpool, zpool, dpool, rpool = env["xpool"], env["zpool"], env["dpool"], env["rpool"]
    apool, opool, gpool = env["apool"], env["opool"], env["gpool"]
    spool = env["spool"]
    psZ, psO = env["psZ"], env["psO"]

    # resident SBUF tiles
    xsb = [xpool.tile([P, 4 * N_TOK], f16, tag="xsb", name=f"xsb{q}") for q in range(XQ)]
    zT = [zpool.tile([P, N_TOK], f16, tag="zT", name=f"zT{m}") for m in range(MT)]
    # decT packed as one tile: decT[kk] = cols [kk*512, kk*512+512)
    decT = dpool.tile([P, KT * R], f16, tag="decT")
    decT_v = decT[:].rearrange("p (s x) -> p s x", x=R)    # [128, 32, 512]

    dec_on = "dec" in phases

    # The rm stream needs ~154 GB/s sustained; one hwdge queue tops out
    # near that, so rm tiles ALTERNATE between the two queues (77 GB/s
    # each). rm[0] is split across both (it gates the first LDWEIGHTS);
    # rm[1] is dispatched ahead of the x chunks; x chunks alternate too
    # so m=0 never waits on a single-queue x backlog.
    rm0 = rpool.tile([P, D], f16, tag="rm")
    rm1 = rpool.tile([P, D], f16, tag="rm")
    nc.sync.dma_start(rm0[:, :D // 2], rtb_v[0][:, :D // 2])
    nc.scalar.dma_start(
        xsb[0][:].rearrange("p (j n) -> p j n", n=N_TOK), xT_v[0])
    nc.scalar.dma_start(rm0[:, D // 2:], rtb_v[0][:, D // 2:])
    nc.sync.dma_start(rm1[:], rtb_v[1])
    for q in range(1, XQ):
        eng = nc.sync if q % 2 == 1 else nc.scalar
        eng.dma_start(
            xsb[q][:].rearrange("p (j n) -> p j n", n=N_TOK), xT_v[q]
        )

    # decode: per row-block i, 4 gathers accumulate into acc via the DMA
    # compute engine; then 4 xbar transposes (e'-quarters) build decT.
    # Each gather's index column is routed through a DVE chain that
    # data-depends on zT[m_g], time-gating gathers to ~1 per 1.7 m-blocks
    # so the rm stream never starves on the DMA engines.
    accs, gscs, gates = [], [], {}
    if dec_on:
        for i in range(RT):
            accs.append(apool.tile([P, D], f16, tag="acc", name=f"acc{i}"))
            gscs.append(spool.tile([P, D // 2], f16, tag="gsc", name=f"gsc{i}"))
        for i in range(RT):
            for k in range(K):
                for h in range(2):
                    gates[(i, k, h)] = (
                        gpool.tile([P, 1], mybir.dt.int32, tag="gate",
                                   name=f"gate{i}_{k}_{h}"),
                        gpool.tile([P, 1], mybir.dt.float32, tag="gatez",
                                   name=f"gatez{i}_{k}_{h}"),
                    )
            for eq in range(4):
                gates[("t", i, eq)] = gpool.tile(
                    [P, 1], mybir.dt.float32, tag="gatez",
                    name=f"gatet{i}_{eq}")
    # decode work queue: per group i, 8 half-gathers (2.9us -> 1.45us DMA
    # bursts) + 4 xbar transposes, drained ~1.8 ops per m-block via the
    # zT gate chain so DMA demand stays smooth next to the rm stream.
    H = D // 2
    decode_ops = []
    for i in range(RT):
        for k in range(K):
            for h in range(2):
                decode_ops.append(("g", i, k, h))
        for eq in range(4):
            decode_ops.append(("t", i, eq))
    gate_at = {}
    for jop, op in enumerate(decode_ops):
        gate_at.setdefault(2 + (27 * jop) // len(decode_ops), []).append(op)

    def emit_decode(op, m):
        if op[0] == "g":
            _, i, k, h = op
            # gated index column: gz = 0*zT[m] (data dep on block m),
            # gt = ridx + gz — the gather can't start before m-block m.
            gt, gz = gates[(i, k, h)]
            nc.vector.tensor_scalar(
                out=gz[:], in0=zT[m][:, 0:1], scalar1=0.0, scalar2=None,
                op0=mybir.AluOpType.mult)
            nc.vector.tensor_scalar(
                out=gt[:], in0=ridx_sb[:, i * K + k: i * K + k + 1],
                scalar1=gz[:], scalar2=None, op0=mybir.AluOpType.add)
            if k == 0:
                nc.gpsimd.indirect_dma_start(
                    out=accs[i][:, h * H:(h + 1) * H], out_offset=None,
                    in_=lcw, element_offset=h * H,
                    in_offset=bass.IndirectOffsetOnAxis(ap=gt[:], axis=0),
                )
            else:
                nc.gpsimd.indirect_dma_start(
                    out=gscs[i][:], out_offset=None,
                    in_=lcw, element_offset=h * H,
                    in_offset=bass.IndirectOffsetOnAxis(ap=gt[:], axis=0),
                )
                nc.vector.tensor_add(
                    accs[i][:, h * H:(h + 1) * H],
                    accs[i][:, h * H:(h + 1) * H],
                    gscs[i][:])
        else:
            # transposes fire the moment acc is ready, so 4 of them burst
            # the DMA engines at every group boundary; time-gate each one
            # with a value-preserving acc[cell] += 0*zT[m] DVE op.
            _, i, eq = op
            qs = eq * (D // 4)
            gz2 = gates[("t", i, eq)]
            nc.vector.tensor_scalar(
                out=gz2[:], in0=zT[m][:, 0:1], scalar1=0.0, scalar2=None,
                op0=mybir.AluOpType.mult)
            nc.vector.tensor_scalar(
                out=accs[i][:, qs:qs + 1], in0=accs[i][:, qs:qs + 1],
                scalar1=gz2[:], scalar2=None, op0=mybir.AluOpType.add)
            nc.scalar.dma_start_transpose(
                decT_v[:, eq * 8:(eq + 1) * 8, i * P:(i + 1) * P],
                accs[i][:, qs:qs + (D // 4)],
            )

    # ---- G1: zT[m] = sum_k rtb[m,:,k-block]^T @ xT[k-block,:] ----
    for m in range(MT):
        if m == 0:
            rm = rm0
        elif m == 1:
            rm = rm1
        else:
            rm = rpool.tile([P, D], f16, tag="rm")
            eng = nc.sync if m % 2 == 0 else nc.scalar
            eng.dma_start(rm[:], rtb_v[m])
        ps = psZ.tile([P, N_TOK], f32, tag="psZ")
        for k in range(KT):
            nc.tensor.matmul(
                ps[:],
                lhsT=rm[:, k * P:(k + 1) * P],
                rhs=xsb[k // 4][:, (k % 4) * N_TOK:(k % 4 + 1) * N_TOK],
                start=(k == 0),
                stop=(k == KT - 1),
            )
        nc.vector.tensor_copy(zT[m][:], ps[:])

        if dec_on:
            for op in gate_at.get(m, []):
                emit_decode(op, m)

    if "g2" not in phases:
        # timing probe (g1 / nog2): consume zT (+ decT if present) so
        # nothing can be DCE'd, skip the G2 GEMM
        out_sb = opool.tile([P, JT * N_TOK], f32, tag="osb")
        for m in range(MT):
            nc.vector.tensor_copy(
                out_sb[:, (m % JT) * N_TOK:(m % JT + 1) * N_TOK], zT[m][:])
        if dec_on:
            for j in range(JT):
                nc.vector.tensor_copy(
                    out_sb[:, j * N_TOK:(j + 1) * N_TOK],
                    decT_v[:, j, :])
        nc.sync.dma_start(
            outT_v, out_sb[:].rearrange("p (j n) -> p j n", n=N_TOK))
        return

    # ---- G2: out[r,n] = sum_e decT[e,r]^T @ zT[e,n] ----
    # j-outer so each output tile's bias+store overlaps the next j's MMs
    out_sb = opool.tile([P, JT * N_TOK], f32, tag="osb")
    for j in range(JT):
        out_ps = psO.tile([P, N_TOK], f32, tag="psO", name=f"outps{j}")
        for m in range(MT):
            nc.tensor.matmul(
                out_ps[:],
                lhsT=decT_v[:, m, j * P:(j + 1) * P],
                rhs=zT[m][:],
                start=(m == 0),
                stop=(m == MT - 1),
            )
        nc.vector.tensor_scalar(
            out=out_sb[:, j * N_TOK:(j + 1) * N_TOK],
            in0=out_ps[:],
            scalar1=bias_sb[:, j:j + 1],
            scalar2=None,
            op0=mybir.AluOpType.add,
        )
        half = N_TOK // 2
        nc.sync.dma_start(
            outT_v[:, j, :half], out_sb[:, j * N_TOK:j * N_TOK + half])
        nc.scalar.dma_start(
            outT_v[:, j, half:], out_sb[:, j * N_TOK + half:(j + 1) * N_TOK])


def _get_program(loop_n=1):
    if loop_n not in _PROGRAM_CACHE:
        _PROGRAM_CACHE[loop_n] = _build_program(loop_n)
    return _PROGRAM_CACHE[loop_n]


def _build_bench_program(loop_n, phases="g1 dec g2"):
    """Timing-only variant: big tensors are Internal (device-resident, no
    per-call upload over axon — kills the transfer noise that swamps the
    loop delta) and the body repeats a static loop_n times. Values in
    lcw/rtb/xT are garbage; engine timing is data-independent. ridx stays
    a real external input (it feeds DMA offsets, which must stay
    in-bounds)."""
    f32 = mybir.dt.float32
    f16 = mybir.dt.float16
    i32 = mybir.dt.int32

    nc = bacc.Bacc("TRN2", target_bir_lowering=False, debug=False)
    lcw = nc.dram_tensor("lcw", [LCW_ROWS, D], f16, kind="Internal").ap()
    ridx = nc.dram_tensor("ridx", [P, RT * K], i32, kind="ExternalInput").ap()
    rtb = nc.dram_tensor("rtb", [MT * P, D], f16, kind="Internal").ap()
    xT = nc.dram_tensor("xT", [D, N_TOK], f16, kind="Internal").ap()
    biasd = nc.dram_tensor("biasd", [P, JT], f32, kind="ExternalInput").ap()
    outT = nc.dram_tensor("outT", [R, N_TOK], f32, kind="ExternalOutput").ap()

    rtb_v = rtb.rearrange("(m p) d -> m p d", p=P)
    xT_v = xT.rearrange("(q j p) n -> q p j n", j=4, p=P)
    outT_v = outT.rearrange("(j p) n -> p j n", p=P)

    with tile.TileContext(nc) as tc:
        with (
            tc.tile_pool(name="const", bufs=1) as cpool,
            tc.tile_pool(name="xbuf", bufs=XQ) as xpool,
            tc.tile_pool(name="zbuf", bufs=MT) as zpool,
            tc.tile_pool(name="decT", bufs=1) as dpool,
            tc.tile_pool(name="rbuf", bufs=5) as rpool,
            tc.tile_pool(name="accp", bufs=RT) as apool,
            tc.tile_pool(name="outp", bufs=1) as opool,
            tc.tile_pool(name="gate", bufs=5 * RT * K) as gpool,
            tc.tile_pool(name="gsc", bufs=RT) as spool,
            tc.tile_pool(name="psZ", bufs=6, space="PSUM") as psZ,
            tc.tile_pool(name="psO", bufs=2, space="PSUM") as psO,
        ):
            ridx_sb = cpool.tile([P, RT * K], i32, tag="ridx_sb")
            nc.gpsimd.dma_start(ridx_sb[:], ridx)
            bias_sb = cpool.tile([P, JT], f32, tag="bias_sb")
            nc.gpsimd.dma_start(bias_sb[:], biasd)

            loop_cm = tc.For_i(0, loop_n, 1) if loop_n > 1 else contextlib.nullcontext()
            with loop_cm:
                _emit_body(nc, tc, locals(), phases=phases)

    nc.compile()
    return nc


def _get_bench_program(loop_n, phases="g1 dec g2"):
    key = ("bench", loop_n, phases)
    if key not in _PROGRAM_CACHE:
        _PROGRAM_CACHE[key] = _build_bench_program(loop_n, phases)
    return _PROGRAM_CACHE[key]


def _make_in_maps(x, codewords, indices, rotations, scales, bias):
    x = np.asarray(x, dtype=np.float32)
    codewords = np.asarray(codewords, dtype=np.float32)
    indices = np.asarray(indices)
    rotations = np.asarray(rotations, dtype=np.float32)
    scales = np.asarray(scales, dtype=np.float32)
    bias = np.asarray(bias, dtype=np.float32)

    xTh = np.ascontiguousarray(x.T.astype(np.float16))  # [4096, 512]
    in_maps = []
    for t in range(T):
        idx_t = indices[t].reshape(-1).astype(np.int64)
        uniq, inv = np.unique(idx_t, return_inverse=True)
        assert len(uniq) <= LCW_ROWS
        lcw = np.zeros((LCW_ROWS, D), np.float16)
        lcw[: len(uniq)] = codewords[uniq].astype(np.float16)
        inv = inv.reshape(R, K).astype(np.int32)
        ridx = np.zeros((P, RT * K), np.int32)
        for i in range(RT):
            for k in range(K):
                ridx[:, i * K + k] = inv[i * P:(i + 1) * P, k]
        # G1 contracts over rot's SECOND axis (e'): z[n,d] = sum_e x[n,e]rot[d,e]
        # lhsT must be rot^T: rtb[m, p, k*128+j] = (rot*s/4)^T[k*128+p, m*128+j]
        rt = (rotations[t] * (scales[t] / K)).T
        rtb = np.ascontiguousarray(
            rt.reshape(KT, P, MT, P).transpose(2, 1, 0, 3)
            .reshape(MT * P, D).astype(np.float16)
        )
        bias_t = np.ascontiguousarray(
            bias[R * t: R * (t + 1)].reshape(JT, P).T
        ).astype(np.float32)
        in_maps.append(
            {"lcw": lcw, "ridx": ridx, "rtb": rtb, "xT": xTh, "biasd": bias_t}
        )
    return in_maps


def kernel(x, codewords, indices, rotations, scales, bias):
    in_maps = _make_in_maps(x, codewords, indices, rotations, scales, bias)
    nc = _get_program()
    res = bass_utils.run_bass_kernel_spmd(nc, in_maps, core_ids=list(range(N_CORES)))
    out = np.empty((N_TOK, T * R), np.float32)
    for t in range(T):
        out[:, R * t: R * (t + 1)] = res.results[t]["outT"].T
    return out


if __name__ == "__main__":
    rng = np.random.default_rng(0)
    ins = {
        "x": rng.standard_normal((N_TOK, D), dtype=np.float32),
        "codewords": rng.standard_normal((N_CW, D), dtype=np.float32) * 0.02,
        "indices": rng.integers(0, N_CW, size=(T, R, K)),
        "rotations": rng.standard_normal((T, D, D), dtype=np.float32) / np.sqrt(D),
        "scales": (rng.random(T, dtype=np.float32) + 0.5),
        "bias": np.zeros(D, np.float32),
    }
    out = kernel(**ins)
    print("out", out.shape, out.dtype, np.abs(out).mean())


# revision 25
# speedup vs baseline: 1.0608x; 1.0496x over previous
"""Bass/Trainium2 SPMD kernel for DemopackDecoder (vq_codebook).

Math: decoded[t] = mean_k codewords[indices[t,:,k]]        [512, 4096]
      W[t]       = (decoded[t] @ rotations[t]) * scales[t] [512, 4096]
      out        = x @ concat_t(W[t]).T + bias             [512, 4096]

Sharding (8 cores, expert-parallel over tiles t): core t computes the
output column block [512 tok, 512 feat] for its tile; host concatenates.

Device dataflow (v4 — PE runs only the two GEMMs; decode rides DMA):
  out_t = x @ rot_t^T @ dec_t^T          (rot pre-scaled by s_t/4 on host)
  G1: zT[d,n] = rot_t^T-blocks @ xT[e',n]  -- rot^T blocks (1MB fp16)
      stream from HBM on the sync queue as the stationary operand; x^T is
      SBUF-resident fp16.  rm[0] leads the sync queue (it gates the first
      LDWEIGHTS, half on each hwdge queue) and x chunks stream on the
      scalar queue in exactly m=0's consumption order, so the PE starts
      ~10us in and never waits for x afterwards.
  A : decode rides DMA + DVE, time-gated behind the rm stream: each
      half-row-block indirect gather (deduped codebook, fp16) reads its
      index column through a DVE chain that data-depends on zT[m_g], so
      gathers release ~1 per 1.7 m-blocks; DVE adds form the mean-of-4;
      DMA xbar transposes (16x128 crossbar, SBUF->SBUF) emit decT
      [e'-part, r] with zero PE involvement. All hidden under G1.
  G2: out[r,n] += decT[e,r-block]^T @ zT[e,n]  (128 MMs, j-outer so each
      output tile's bias+store overlaps the next chain)
  C : + bias (DVE, PSUM-read), stores split across both hwdge queues.
All matmul operands fp16 (1 PE cycle/row, ~215ns/MM measured = stream
roofline); PSUM fp32; psZ 6 banks absorb zT-copy jitter, psO 2.
Measured ~290-310us on HW (NTFF) vs 444775ns baseline.
"""

import contextlib

import numpy as np

import concourse.bass as bass
import concourse.mybir as mybir
import concourse.tile as tile
from concourse import bacc, bass_utils

# Problem shapes (hardcoded per contract)
T, R, K, D = 8, 512, 4, 4096
N_CW, N_TOK, N_CORES = 16384, 512, 8
P = 128
LCW_ROWS = 2048          # padded dedup codebook rows per core
KT = D // P              # 32 contraction (e') tiles
MT = D // P              # 32 rotated-feature (d) tiles
RT = R // P              # 4 decoded row tiles
JT = R // P              # 4 local out-feature tiles
XQ = 8                   # x loaded in 8 chunks of 4 k-blocks

_PROGRAM_CACHE = {}


def _build_program(loop_n=1):
    f32 = mybir.dt.float32
    f16 = mybir.dt.float16
    i32 = mybir.dt.int32

    nc = bacc.Bacc("TRN2", target_bir_lowering=False, debug=False)
    lcw = nc.dram_tensor("lcw", [LCW_ROWS, D], f16, kind="ExternalInput").ap()
    ridx = nc.dram_tensor("ridx", [P, RT * K], i32, kind="ExternalInput").ap()
    rtb = nc.dram_tensor("rtb", [MT * P, D], f16, kind="ExternalInput").ap()
    xT = nc.dram_tensor("xT", [D, N_TOK], f16, kind="ExternalInput").ap()
    biasd = nc.dram_tensor("biasd", [P, JT], f32, kind="ExternalInput").ap()
    outT = nc.dram_tensor("outT", [R, N_TOK], f32, kind="ExternalOutput").ap()

    # DRAM views
    rtb_v = rtb.rearrange("(m p) d -> m p d", p=P)         # [32, 128, 4096]
    xT_v = xT.rearrange("(q j p) n -> q p j n", j=4, p=P)  # [8, 128, 4, 512]
    outT_v = outT.rearrange("(j p) n -> p j n", p=P)       # [128, 4, 512]

    with tile.TileContext(nc) as tc:
        with (
            tc.tile_pool(name="const", bufs=1) as cpool,
            tc.tile_pool(name="xbuf", bufs=XQ) as xpool,
            tc.tile_pool(name="zbuf", bufs=MT) as zpool,
            tc.tile_pool(name="decT", bufs=1) as dpool,
            tc.tile_pool(name="rbuf", bufs=8) as rpool,
            tc.tile_pool(name="accp", bufs=2) as apool,
            tc.tile_pool(name="outp", bufs=1) as opool,
            tc.tile_pool(name="gate", bufs=5 * RT * K) as gpool,
            tc.tile_pool(name="gsc", bufs=2) as spool,
            tc.tile_pool(name="psZ", bufs=6, space="PSUM") as psZ,
            tc.tile_pool(name="psO", bufs=2, space="PSUM") as psO,
        ):
            ridx_sb = cpool.tile([P, RT * K], i32, tag="ridx_sb")
            nc.gpsimd.dma_start(ridx_sb[:], ridx)
            bias_sb = cpool.tile([P, JT], f32, tag="bias_sb")
            nc.gpsimd.dma_start(bias_sb[:], biasd)

            loop_cm = tc.For_i(0, loop_n, 1) if loop_n > 1 else contextlib.nullcontext()
            with loop_cm:
                _emit_body(nc, tc, locals())

    nc.compile()
    return nc


def _emit_body(nc, tc, env, phases="g1 dec g2"):
    f32 = mybir.dt.float32
    f16 = mybir.dt.float16
    lcw, ridx_sb, bias_sb = env["lcw"], env["ridx_sb"], env["bias_sb"]
    rtb_v, xT_v, outT_v = env["rtb_v"], env["xT_v"], env["outT_v"]
    xpool, zpool, dpool, rpool = env["xpool"], env["zpool"], env["dpool"], env["rpool"]
    apool, opool, gpool = env["apool"], env["opool"], env["gpool"]
    spool = env["spool"]
    psZ, psO = env["psZ"], env["psO"]

    # resident SBUF tiles
    xsb = [xpool.tile([P, 4 * N_TOK], f16, tag="xsb", name=f"xsb{q}") for q in range(XQ)]
    zT = [zpool.tile([P, N_TOK], f16, tag="zT", name=f"zT{m}") for m in range(MT)]
    # decT packed as one tile: decT[kk] = cols [kk*512, kk*512+512)
    decT = dpool.tile([P, KT * R], f16, tag="decT")
    decT_v = decT[:].rearrange("p (s x) -> p s x", x=R)    # [128, 32, 512]

    dec_on = "dec" in phases

    # The rm stream needs ~154 GB/s sustained; one hwdge queue tops out
    # near that, so rm tiles ALTERNATE between the two queues (77 GB/s
    # each). rm[0] is split across both (it gates the first LDWEIGHTS);
    # rm[1] is dispatched ahead of the x chunks; x chunks alternate too
    # so m=0 never waits on a single-queue x backlog.
    rm0 = rpool.tile([P, D], f16, tag="rm")
    rm1 = rpool.tile([P, D], f16, tag="rm")
    nc.sync.dma_start(rm0[:, :D // 2], rtb_v[0][:, :D // 2])
    nc.scalar.dma_start(
        xsb[0][:].rearrange("p (j n) -> p j n", n=N_TOK), xT_v[0])
    nc.scalar.dma_start(rm0[:, D // 2:], rtb_v[0][:, D // 2:])
    nc.sync.dma_start(rm1[:], rtb_v[1])
    for q in range(1, XQ):
        eng = nc.sync if q % 2 == 1 else nc.scalar
        eng.dma_start(
            xsb[q][:].rearrange("p (j n) -> p j n", n=N_TOK), xT_v[q]
        )

    # decode: per row-block i, 4 gathers accumulate into acc via the DMA
    # compute engine; then 4 xbar transposes (e'-quarters) build decT.
    # Each gather's index column is routed through a DVE chain that
    # data-depends on zT[m_g], time-gating gathers to ~1 per 1.7 m-blocks
    # so the rm stream never starves on the DMA engines.
    accs, gscs, gates = [], [], {}
    if dec_on:
        for i in range(RT):
            accs.append(apool.tile([P, D], f16, tag="acc", name=f"acc{i}"))
            gscs.append(spool.tile([P, D // 2], f16, tag="gsc", name=f"gsc{i}"))
        for i in range(RT):
            for k in range(K):
                for h in range(2):
                    gates[(i, k, h)] = (
                        gpool.tile([P, 1], mybir.dt.int32, tag="gate",
                                   name=f"gate{i}_{k}_{h}"),
                        gpool.tile([P, 1], mybir.dt.float32, tag="gatez",
                                   name=f"gatez{i}_{k}_{h}"),
                    )
            for eq in range(4):
                gates[("t", i, eq)] = gpool.tile(
                    [P, 1], mybir.dt.float32, tag="gatez",
                    name=f"gatet{i}_{eq}")
    # decode work queue: per group i, 8 half-gathers (2.9us -> 1.45us DMA
    # bursts) + 4 xbar transposes, drained ~1.8 ops per m-block via the
    # zT gate chain so DMA demand stays smooth next to the rm stream.
    H = D // 2
    decode_ops = []
    for i in range(RT):
        for k in range(K):
            for h in range(2):
                decode_ops.append(("g", i, k, h))
        for eq in range(4):
            decode_ops.append(("t", i, eq))
    gate_at = {}
    for jop, op in enumerate(decode_ops):
        gate_at.setdefault(2 + (27 * jop) // len(decode_ops), []).append(op)

    def emit_decode(op, m):
        if op[0] == "g":
            _, i, k, h = op
            # gated index column: gz = 0*zT[m] (data dep on block m),
            # gt = ridx + gz — the gather can't start before m-block m.
            gt, gz = gates[(i, k, h)]
            nc.vector.tensor_scalar(
                out=gz[:], in0=zT[m][:, 0:1], scalar1=0.0, scalar2=None,
                op0=mybir.AluOpType.mult)
            nc.vector.tensor_scalar(
                out=gt[:], in0=ridx_sb[:, i * K + k: i * K + k + 1],
                scalar1=gz[:], scalar2=None, op0=mybir.AluOpType.add)
            if k == 0:
                nc.gpsimd.indirect_dma_start(
                    out=accs[i][:, h * H:(h + 1) * H], out_offset=None,
                    in_=lcw, element_offset=h * H,
                    in_offset=bass.IndirectOffsetOnAxis(ap=gt[:], axis=0),
                )
            else:
                nc.gpsimd.indirect_dma_start(
                    out=gscs[i][:], out_offset=None,
                    in_=lcw, element_offset=h * H,
                    in_offset=bass.IndirectOffsetOnAxis(ap=gt[:], axis=0),
                )
                nc.vector.tensor_add(
                    accs[i][:, h * H:(h + 1) * H],
                    accs[i][:, h * H:(h + 1) * H],
                    gscs[i][:])
        else:
            # transposes fire the moment acc is ready, so 4 of them burst
            # the DMA engines at every group boundary; time-gate each one
            # with a value-preserving acc[cell] += 0*zT[m] DVE op.
            _, i, eq = op
            qs = eq * (D // 4)
            gz2 = gates[("t", i, eq)]
            nc.vector.tensor_scalar(
                out=gz2[:], in0=zT[m][:, 0:1], scalar1=0.0, scalar2=None,
                op0=mybir.AluOpType.mult)
            nc.vector.tensor_scalar(
                out=accs[i][:, qs:qs + 1], in0=accs[i][:, qs:qs + 1],
                scalar1=gz2[:], scalar2=None, op0=mybir.AluOpType.add)
            nc.scalar.dma_start_transpose(
                decT_v[:, eq * 8:(eq + 1) * 8, i * P:(i + 1) * P],
                accs[i][:, qs:qs + (D // 4)],
            )

    # ---- G1: zT[m] = sum_k rtb[m,:,k-block]^T @ xT[k-block,:] ----
    for m in range(MT):
        if m == 0:
            rm = rm0
        elif m == 1:
            rm = rm1
        else:
            rm = rpool.tile([P, D], f16, tag="rm")
            eng = nc.sync if m % 2 == 0 else nc.scalar
            eng.dma_start(rm[:], rtb_v[m])
        ps = psZ.tile([P, N_TOK], f32, tag="psZ")
        for k in range(KT):
            nc.tensor.matmul(
                ps[:],
                lhsT=rm[:, k * P:(k + 1) * P],
                rhs=xsb[k // 4][:, (k % 4) * N_TOK:(k % 4 + 1) * N_TOK],
                start=(k == 0),
                stop=(k == KT - 1),
            )
        nc.vector.tensor_copy(zT[m][:], ps[:])

        if dec_on:
            for op in gate_at.get(m, []):
                emit_decode(op, m)

    if "g2" not in phases:
        # timing probe (g1 / nog2): consume zT (+ decT if present) so
        # nothing can be DCE'd, skip the G2 GEMM
        out_sb = opool.tile([P, JT * N_TOK], f32, tag="osb")
        for m in range(MT):
            nc.vector.tensor_copy(
                out_sb[:, (m % JT) * N_TOK:(m % JT + 1) * N_TOK], zT[m][:])
        if dec_on:
            for j in range(JT):
                nc.vector.tensor_copy(
                    out_sb[:, j * N_TOK:(j + 1) * N_TOK],
                    decT_v[:, j, :])
        nc.sync.dma_start(
            outT_v, out_sb[:].rearrange("p (j n) -> p j n", n=N_TOK))
        return

    # ---- G2: out[r,n] = sum_e decT[e,r]^T @ zT[e,n] ----
    # j-outer so each output tile's bias+store overlaps the next j's MMs
    out_sb = opool.tile([P, JT * N_TOK], f32, tag="osb")
    for j in range(JT):
        out_ps = psO.tile([P, N_TOK], f32, tag="psO", name=f"outps{j}")
        for m in range(MT):
            nc.tensor.matmul(
                out_ps[:],
                lhsT=decT_v[:, m, j * P:(j + 1) * P],
                rhs=zT[m][:],
                start=(m == 0),
                stop=(m == MT - 1),
            )
        nc.vector.tensor_scalar(
            out=out_sb[:, j * N_TOK:(j + 1) * N_TOK],
            in0=out_ps[:],
            scalar1=bias_sb[:, j:j + 1],
            scalar2=None,
            op0=mybir.AluOpType.add,
        )
        half = N_TOK // 2
        nc.sync.dma_start(
            outT_v[:, j, :half], out_sb[:, j * N_TOK:j * N_TOK + half])
        nc.scalar.dma_start(
            outT_v[:, j, half:], out_sb[:, j * N_TOK + half:(j + 1) * N_TOK])


def _get_program(loop_n=1):
    if loop_n not in _PROGRAM_CACHE:
        _PROGRAM_CACHE[loop_n] = _build_program(loop_n)
    return _PROGRAM_CACHE[loop_n]


def _build_bench_program(loop_n, phases="g1 dec g2"):
    """Timing-only variant: big tensors are Internal (device-resident, no
    per-call upload over axon — kills the transfer noise that swamps the
    loop delta) and the body repeats a static loop_n times. Values in
    lcw/rtb/xT are garbage; engine timing is data-independent. ridx stays
    a real external input (it feeds DMA offsets, which must stay
    in-bounds)."""
    f32 = mybir.dt.float32
    f16 = mybir.dt.float16
    i32 = mybir.dt.int32

    nc = bacc.Bacc("TRN2", target_bir_lowering=False, debug=False)
    lcw = nc.dram_tensor("lcw", [LCW_ROWS, D], f16, kind="Internal").ap()
    ridx = nc.dram_tensor("ridx", [P, RT * K], i32, kind="ExternalInput").ap()
    rtb = nc.dram_tensor("rtb", [MT * P, D], f16, kind="Internal").ap()
    xT = nc.dram_tensor("xT", [D, N_TOK], f16, kind="Internal").ap()
    biasd = nc.dram_tensor("biasd", [P, JT], f32, kind="ExternalInput").ap()
    outT = nc.dram_tensor("outT", [R, N_TOK], f32, kind="ExternalOutput").ap()

    rtb_v = rtb.rearrange("(m p) d -> m p d", p=P)
    xT_v = xT.rearrange("(q j p) n -> q p j n", j=4, p=P)
    outT_v = outT.rearrange("(j p) n -> p j n", p=P)

    with tile.TileContext(nc) as tc:
        with (
            tc.tile_pool(name="const", bufs=1) as cpool,
            tc.tile_pool(name="xbuf", bufs=XQ) as xpool,
            tc.tile_pool(name="zbuf", bufs=MT) as zpool,
            tc.tile_pool(name="decT", bufs=1) as dpool,
            tc.tile_pool(name="rbuf", bufs=8) as rpool,
            tc.tile_pool(name="accp", bufs=2) as apool,
            tc.tile_pool(name="outp", bufs=1) as opool,
            tc.tile_pool(name="gate", bufs=5 * RT * K) as gpool,
            tc.tile_pool(name="gsc", bufs=2) as spool,
            tc.tile_pool(name="psZ", bufs=6, space="PSUM") as psZ,
            tc.tile_pool(name="psO", bufs=2, space="PSUM") as psO,
        ):
            ridx_sb = cpool.tile([P, RT * K], i32, tag="ridx_sb")
            nc.gpsimd.dma_start(ridx_sb[:], ridx)
            bias_sb = cpool.tile([P, JT], f32, tag="bias_sb")
            nc.gpsimd.dma_start(bias_sb[:], biasd)

            loop_cm = tc.For_i(0, loop_n, 1) if loop_n > 1 else contextlib.nullcontext()
            with loop_cm:
                _emit_body(nc, tc, locals(), phases=phases)

    nc.compile()
    return nc


def _get_bench_program(loop_n, phases="g1 dec g2"):
    key = ("bench", loop_n, phases)
    if key not in _PROGRAM_CACHE:
        _PROGRAM_CACHE[key] = _build_bench_program(loop_n, phases)
    return _PROGRAM_CACHE[key]


def _make_in_maps(x, codewords, indices, rotations, scales, bias):
    x = np.asarray(x, dtype=np.float32)
    codewords = np.asarray(codewords, dtype=np.float32)
    indices = np.asarray(indices)
    rotations = np.asarray(rotations, dtype=np.float32)
    scales = np.asarray(scales, dtype=np.float32)
    bias = np.asarray(bias, dtype=np.float32)

    xTh = np.ascontiguousarray(x.T.astype(np.float16))  # [4096, 512]
    in_maps = []
    for t in range(T):
        idx_t = indices[t].reshape(-1).astype(np.int64)
        uniq, inv = np.unique(idx_t, return_inverse=True)
        assert len(uniq) <= LCW_ROWS
        lcw = np.zeros((LCW_ROWS, D), np.float16)
        lcw[: len(uniq)] = codewords[uniq].astype(np.float16)
        inv = inv.reshape(R, K).astype(np.int32)
        ridx = np.zeros((P, RT * K), np.int32)
        for i in range(RT):
            for k in range(K):
                ridx[:, i * K + k] = inv[i * P:(i + 1) * P, k]
        # G1 contracts over rot's SECOND axis (e'): z[n,d] = sum_e x[n,e]rot[d,e]
        # lhsT must be rot^T: rtb[m, p, k*128+j] = (rot*s/4)^T[k*128+p, m*128+j]
        rt = (rotations[t] * (scales[t] / K)).T
        rtb = np.ascontiguousarray(
            rt.reshape(KT, P, MT, P).transpose(2, 1, 0, 3)
            .reshape(MT * P, D).astype(np.float16)
        )
        bias_t = np.ascontiguousarray(
            bias[R * t: R * (t + 1)].reshape(JT, P).T
        ).astype(np.float32)
        in_maps.append(
            {"lcw": lcw, "ridx": ridx, "rtb": rtb, "xT": xTh, "biasd": bias_t}
        )
    return in_maps


def kernel(x, codewords, indices, rotations, scales, bias):
    in_maps = _make_in_maps(x, codewords, indices, rotations, scales, bias)
    nc = _get_program()
    res = bass_utils.run_bass_kernel_spmd(nc, in_maps, core_ids=list(range(N_CORES)))
    out = np.empty((N_TOK, T * R), np.float32)
    for t in range(T):
        out[:, R * t: R * (t + 1)] = res.results[t]["outT"].T
    return out


if __name__ == "__main__":
    rng = np.random.default_rng(0)
    ins = {
        "x": rng.standard_normal((N_TOK, D), dtype=np.float32),
        "codewords": rng.standard_normal((N_CW, D), dtype=np.float32) * 0.02,
        "indices": rng.integers(0, N_CW, size=(T, R, K)),
        "rotations": rng.standard_normal((T, D, D), dtype=np.float32) / np.sqrt(D),
        "scales": (rng.random(T, dtype=np.float32) + 0.5),
        "bias": np.zeros(D, np.float32),
    }
    out = kernel(**ins)
    print("out", out.shape, out.dtype, np.abs(out).mean())
